# revision 24
# baseline (speedup 1.0000x reference)
"""AttentionFusion Trainium2 kernel.

Computes, for row-major x1, x2 [N, 128] and weight vector w [128]:
    s1 = x1 @ w ; s2 = x2 @ w
    alpha1 = sigmoid(s1 - s2) ; alpha2 = 1 - alpha1
    fused = alpha1[:, None] * x1 + alpha2[:, None] * x2
Returns (fused [N, 128] f32, alpha [N, 2] f32).

Data-parallel over 8 NeuronCores: rows split evenly, w replicated (w is
pre-tiled host-side to [128, G*128] so the elementwise multiply needs no
broadcast). Per-core bass/Tile program, per [128, G*128] tile (row = p*G + g
so every tile is one contiguous DRAM block):
    DMA : load x2 ; accumulate-load x1 (SWDGE CCE add) onto -x2 giving t
    DVE : -x2 (2x-mode tensor_scalar) ; p = t * wb ; grouped reduce -> s ;
          quarter-split fused adds f1 += x2 (software-pipelined 2 tiles back)
    ACT : a = sigmoid(s) ; 16 per-group scale-copies f1_g = t_g * a_g ;
          alpha interleave (a, 1-a)
Measured on HW: 362.8 us/core best, ~363-377 us across runs (~91% DMA-engine
occupancy; memory-bound).
Notes from HW bring-up: custom-ucode DVE ops (tensor_tensor_reduce) crash the
device on this execution path; TensorScalarPtr (scalar_tensor_tensor /
tensor_scalar with an AP scalar) returns wrong data on HW; zero-stride
broadcast APs on DVE tensor_tensor return wrong data; DMA CCE supports add
(not subtract); Pool elementwise ops steal the DVE's shared SBUF port ~1:1 so
Pool offload never helps while DVE has work. ACT activation(Copy, scale=AP)
is verified good and is the only per-partition-scalar multiply that works.
"""

import os
import sys

import numpy as np


def _ensure_concourse():
    try:
        import concourse.bass  # noqa: F401
        return
    except ImportError:
        pass
    for p in ("/opt/trn_rl_repo", os.path.expanduser("~/.axon_site/_ro/trn_rl_repo")):
        if os.path.isdir(p) and p not in sys.path:
            sys.path.insert(0, p)
    import concourse.bass  # noqa: F401


N_FULL = 524288
D = 128
NCORES = 8
NC_ROWS = N_FULL // NCORES  # 65536 rows per core
P = 128                     # SBUF partitions
G = 16                      # rows per partition per tile
ROWS_PER_TILE = P * G       # 2048
F = G * D                   # free-dim elements per tile


def build_program(nc_rows=NC_ROWS):
    """Build the single-core bass program for `nc_rows` rows."""
    _ensure_concourse()
    import concourse.bacc as bacc
    import concourse.tile as tile
    import concourse.mybir as mybir

    f32 = mybir.dt.float32
    AFT = mybir.ActivationFunctionType
    add = mybir.AluOpType.add
    ntiles = nc_rows // ROWS_PER_TILE
    assert nc_rows % ROWS_PER_TILE == 0

    nc = bacc.Bacc(
        "TRN2", target_bir_lowering=False, debug=False, enable_asserts=False
    )
    x1 = nc.dram_tensor("x1", [nc_rows, D], f32, kind="ExternalInput").ap()
    x2 = nc.dram_tensor("x2", [nc_rows, D], f32, kind="ExternalInput").ap()
    wb = nc.dram_tensor("wb", [P, F], f32, kind="ExternalInput").ap()
    fused = nc.dram_tensor("fused", [nc_rows, D], f32, kind="ExternalOutput").ap()
    alpha = nc.dram_tensor("alpha", [nc_rows, 2], f32, kind="ExternalOutput").ap()

    LAG = 2  # stage-B (fused add + store) trails stage A by this many tiles

    with tile.TileContext(nc) as tc:
        with (
            tc.tile_pool(name="wpool", bufs=1) as wpool,
            tc.tile_pool(name="x2p", bufs=6) as x2p,
            tc.tile_pool(name="x1p", bufs=3) as x1p,
            tc.tile_pool(name="mid", bufs=5) as mid,
            tc.tile_pool(name="f1p", bufs=5) as f1p,
            tc.tile_pool(name="sm", bufs=8) as sm,
        ):
            wt = wpool.tile([P, F], f32)
            nc.sync.dma_start(out=wt[:], in_=wb)

            # Software pipeline: stage B (fused add + store) of tile i-LAG is
            # emitted inside tile i's stage A, giving ACT LAG full cycles to
            # finish the scale-copies before DVE's in-order add needs them.
            pending = []  # (f1, x2t, r0, r1) of previous iterations
            for i in range(ntiles + LAG):
                if len(pending) == LAG or (i >= ntiles and pending):
                    pf1, px2t, pr0, pr1 = pending.pop(0)
                    # Quarter-split adds: each depends on only 4 of the 16
                    # scale-copies, so the DVE never waits on the full ACT chain.
                    for q in range(4):
                        qs = slice(q * (F // 4), (q + 1) * (F // 4))
                        nc.vector.tensor_add(pf1[:, qs], pf1[:, qs], px2t[:, qs])
                    nc.sync.dma_start(
                        out=fused[pr0:pr1, :].rearrange("(p g) d -> p (g d)", p=P),
                        in_=pf1[:],
                    )
                if i < ntiles:
                    r0, r1 = i * ROWS_PER_TILE, (i + 1) * ROWS_PER_TILE
                    x2t = x2p.tile([P, F], f32)
                    nc.sync.dma_start(
                        out=x2t[:],
                        in_=x2[r0:r1, :].rearrange("(p g) d -> p (g d)", p=P),
                    )

                    # t = x1 - x2, alternating two full-tile strategies to
                    # balance DMA-engine time vs DVE time: the CCE accumulate
                    # costs ~1.9x plain DMA, the DVE subtract costs ~2x the
                    # 2x-mode negate-copy. Even tiles: negate x2 into t and
                    # accumulate-load x1 onto it (SWDGE CCE add). Odd tiles:
                    # plain x1 load + DVE subtract.
                    t = mid.tile([P, F], f32)
                    if i % 2 == 0:
                        nc.vector.tensor_scalar_mul(t[:], x2t[:], -1.0)
                        nc.gpsimd.dma_start(
                            out=t[:],
                            in_=x1[r0:r1, :].rearrange("(p g) d -> p (g d)", p=P),
                            accum_op=add,
                        )
                    else:
                        x1t = x1p.tile([P, F], f32)
                        nc.sync.dma_start(
                            out=x1t[:],
                            in_=x1[r0:r1, :].rearrange("(p g) d -> p (g d)", p=P),
                        )
                        nc.vector.tensor_sub(t[:], x1t[:], x2t[:])
                    pbig = mid.tile([P, F], f32)
                    nc.vector.tensor_mul(pbig[:], t[:], wt[:])
                    s = sm.tile([P, G], f32)
                    nc.vector.tensor_reduce(
                        out=s[:],
                        in_=pbig[:].rearrange("p (g d) -> p g d", g=G),
                        axis=mybir.AxisListType.X,
                        op=add,
                    )

                    a = sm.tile([P, G], f32)
                    nc.scalar.activation(a[:], s[:], AFT.Sigmoid)

                    # f1_g = t_g * a_g (per-partition scale on ACT) — emitted
                    # straight after sigmoid so the fused adds unblock early.
                    f1 = f1p.tile([P, F], f32)
                    for g in range(G):
                        gs = slice(g * D, (g + 1) * D)
                        nc.scalar.activation(
                            f1[:, gs], t[:, gs], AFT.Copy, scale=a[:, g : g + 1]
                        )

                    # alpha tile interleaved (a, 1-a) -> [P, G*2]
                    al = sm.tile([P, 2 * G], f32)
                    al_v = al[:].rearrange("p (g c) -> p c g", c=2)
                    nc.scalar.copy(al_v[:, 0], a[:])
                    nc.scalar.activation(
                        al_v[:, 1], a[:], AFT.Copy, bias=1.0, scale=-1.0
                    )
                    nc.sync.dma_start(
                        out=alpha[r0:r1, :].rearrange("(p g) c -> p (g c)", p=P),
                        in_=al[:],
                    )

                if i < ntiles:
                    pending.append((f1, x2t, r0, r1))

    nc.compile()
    return nc


_program_cache = {}


def _get_program(nc_rows=NC_ROWS):
    if nc_rows not in _program_cache:
        _program_cache[nc_rows] = build_program(nc_rows)
    return _program_cache[nc_rows]


def make_in_maps(x1, x2, attention_weights):
    x1 = np.ascontiguousarray(np.asarray(x1, dtype=np.float32))
    x2 = np.ascontiguousarray(np.asarray(x2, dtype=np.float32))
    w = np.asarray(attention_weights, dtype=np.float32)
    wb = np.ascontiguousarray(np.tile(w[None, :], (P, G)))
    x1s = np.split(x1, NCORES, axis=0)
    x2s = np.split(x2, NCORES, axis=0)
    return [
        {"x1": np.ascontiguousarray(x1s[c]), "x2": np.ascontiguousarray(x2s[c]), "wb": wb}
        for c in range(NCORES)
    ]


def run(x1, x2, attention_weights, trace=False, **trace_kwargs):
    """Run on 8 cores; returns ((fused, alpha), BassKernelResults)."""
    _ensure_concourse()
    from concourse.bass_utils import run_bass_kernel_spmd

    nc = _get_program()
    in_maps = make_in_maps(x1, x2, attention_weights)
    res = run_bass_kernel_spmd(
        nc, in_maps, core_ids=list(range(NCORES)), trace=trace, **trace_kwargs
    )
    fused = np.concatenate([res.results[c]["fused"] for c in range(NCORES)], axis=0)
    alpha = np.concatenate([res.results[c]["alpha"] for c in range(NCORES)], axis=0)
    return (fused, alpha), res


def kernel(x1, x2, attention_weights):
    (fused, alpha), _ = run(x1, x2, attention_weights, trace=False)
    return fused, alpha


# revision 25
# speedup vs baseline: 1.1343x; 1.1343x over previous
"""AttentionFusion Trainium2 kernel.

Computes, for row-major x1, x2 [N, 128] and weight vector w [128]:
    s1 = x1 @ w ; s2 = x2 @ w
    alpha1 = sigmoid(s1 - s2) ; alpha2 = 1 - alpha1
    fused = alpha1[:, None] * x1 + alpha2[:, None] * x2
Returns (fused [N, 128] f32, alpha [N, 2] f32).

Data-parallel over 8 NeuronCores: rows split evenly, w replicated (w is
pre-tiled host-side to [128, G*128] so the elementwise multiply needs no
broadcast). Per-core bass/Tile program, per [128, G*128] tile (row = p*G + g
so every tile is one contiguous DRAM block):
    DMA : load x2 ; accumulate-load x1 (SWDGE CCE add) onto -x2 giving t
    DVE : -x2 (2x-mode tensor_scalar) ; p = t * wb ; grouped reduce -> s ;
          quarter-split fused adds f1 += x2 (software-pipelined 2 tiles back)
    ACT : a = sigmoid(s) ; 16 per-group scale-copies f1_g = t_g * a_g ;
          alpha interleave (a, 1-a)
Measured on HW: 362.8 us/core best, ~363-377 us across runs (~91% DMA-engine
occupancy; memory-bound).
Notes from HW bring-up: custom-ucode DVE ops (tensor_tensor_reduce) crash the
device on this execution path; TensorScalarPtr (scalar_tensor_tensor /
tensor_scalar with an AP scalar) returns wrong data on HW; zero-stride
broadcast APs on DVE tensor_tensor return wrong data; DMA CCE supports add
(not subtract); Pool elementwise ops steal the DVE's shared SBUF port ~1:1 so
Pool offload never helps while DVE has work. ACT activation(Copy, scale=AP)
is verified good and is the only per-partition-scalar multiply that works.
"""

import os
import sys

import numpy as np


def _ensure_concourse():
    try:
        import concourse.bass  # noqa: F401
        return
    except ImportError:
        pass
    for p in ("/opt/trn_rl_repo", os.path.expanduser("~/.axon_site/_ro/trn_rl_repo")):
        if os.path.isdir(p) and p not in sys.path:
            sys.path.insert(0, p)
    import concourse.bass  # noqa: F401


N_FULL = 524288
D = 128
NCORES = 8
NC_ROWS = N_FULL // NCORES  # 65536 rows per core
P = 128                     # SBUF partitions
G = 16                      # rows per partition per tile
ROWS_PER_TILE = P * G       # 2048
F = G * D                   # free-dim elements per tile


def build_program(nc_rows=NC_ROWS):
    """Build the single-core bass program for `nc_rows` rows."""
    _ensure_concourse()
    import concourse.bacc as bacc
    import concourse.tile as tile
    import concourse.mybir as mybir

    f32 = mybir.dt.float32
    AFT = mybir.ActivationFunctionType
    add = mybir.AluOpType.add
    ntiles = nc_rows // ROWS_PER_TILE
    assert nc_rows % ROWS_PER_TILE == 0

    nc = bacc.Bacc(
        "TRN2", target_bir_lowering=False, debug=False, enable_asserts=False
    )
    x1 = nc.dram_tensor("x1", [nc_rows, D], f32, kind="ExternalInput").ap()
    x2 = nc.dram_tensor("x2", [nc_rows, D], f32, kind="ExternalInput").ap()
    wb = nc.dram_tensor("wb", [P, F], f32, kind="ExternalInput").ap()
    fused = nc.dram_tensor("fused", [nc_rows, D], f32, kind="ExternalOutput").ap()
    alpha = nc.dram_tensor("alpha", [nc_rows, 2], f32, kind="ExternalOutput").ap()

    LAG = 2  # stage-B (fused add + store) trails stage A by this many tiles

    with tile.TileContext(nc) as tc:
        with (
            tc.tile_pool(name="wpool", bufs=1) as wpool,
            tc.tile_pool(name="x2p", bufs=7) as x2p,
            tc.tile_pool(name="mid", bufs=5) as mid,
            tc.tile_pool(name="f1p", bufs=5) as f1p,
            tc.tile_pool(name="sm", bufs=8) as sm,
        ):
            wt = wpool.tile([P, F], f32)
            nc.sync.dma_start(out=wt[:], in_=wb)

            # Software pipeline: stage B (fused add + store) of tile i-LAG is
            # emitted inside tile i's stage A, giving ACT LAG full cycles to
            # finish the scale-copies before DVE's in-order add needs them.
            pending = []  # (f1, x2t, r0, r1) of previous iterations
            for i in range(ntiles + LAG):
                if len(pending) == LAG or (i >= ntiles and pending):
                    pf1, px2t, pr0, pr1 = pending.pop(0)
                    # Quarter-split adds: each depends on only 4 of the 16
                    # scale-copies, so the DVE never waits on the full ACT chain.
                    for q in range(4):
                        qs = slice(q * (F // 4), (q + 1) * (F // 4))
                        nc.vector.tensor_add(pf1[:, qs], pf1[:, qs], px2t[:, qs])
                    nc.sync.dma_start(
                        out=fused[pr0:pr1, :].rearrange("(p g) d -> p (g d)", p=P),
                        in_=pf1[:],
                    )
                if i < ntiles:
                    r0, r1 = i * ROWS_PER_TILE, (i + 1) * ROWS_PER_TILE
                    x2t = x2p.tile([P, F], f32)
                    nc.sync.dma_start(
                        out=x2t[:],
                        in_=x2[r0:r1, :].rearrange("(p g) d -> p (g d)", p=P),
                    )

                    # t = x1 - x2 without a DVE subtract: negate x2 into t via
                    # 2x-mode tensor_scalar, then accumulate-load x1 onto it
                    # (SWDGE CCE add) so the x1 load itself performs the add.
                    t = mid.tile([P, F], f32)
                    nc.vector.tensor_scalar_mul(t[:], x2t[:], -1.0)
                    nc.gpsimd.dma_start(
                        out=t[:],
                        in_=x1[r0:r1, :].rearrange("(p g) d -> p (g d)", p=P),
                        accum_op=add,
                    )
                    pbig = mid.tile([P, F], f32)
                    nc.vector.tensor_mul(pbig[:], t[:], wt[:])
                    s = sm.tile([P, G], f32)
                    nc.vector.tensor_reduce(
                        out=s[:],
                        in_=pbig[:].rearrange("p (g d) -> p g d", g=G),
                        axis=mybir.AxisListType.X,
                        op=add,
                    )

                    a = sm.tile([P, G], f32)
                    nc.scalar.activation(a[:], s[:], AFT.Sigmoid)

                    # f1_g = t_g * a_g (per-partition scale on ACT) — emitted
                    # straight after sigmoid so the fused adds unblock early.
                    f1 = f1p.tile([P, F], f32)
                    for g in range(G):
                        gs = slice(g * D, (g + 1) * D)
                        nc.scalar.activation(
                            f1[:, gs], t[:, gs], AFT.Copy, scale=a[:, g : g + 1]
                        )

                    # alpha tile interleaved (a, 1-a) -> [P, G*2]
                    al = sm.tile([P, 2 * G], f32)
                    al_v = al[:].rearrange("p (g c) -> p c g", c=2)
                    nc.scalar.copy(al_v[:, 0], a[:])
                    nc.scalar.activation(
                        al_v[:, 1], a[:], AFT.Copy, bias=1.0, scale=-1.0
                    )
                    nc.sync.dma_start(
                        out=alpha[r0:r1, :].rearrange("(p g) c -> p (g c)", p=P),
                        in_=al[:],
                    )

                if i < ntiles:
                    pending.append((f1, x2t, r0, r1))

    nc.compile()
    return nc


_program_cache = {}


def _get_program(nc_rows=NC_ROWS):
    if nc_rows not in _program_cache:
        _program_cache[nc_rows] = build_program(nc_rows)
    return _program_cache[nc_rows]


def make_in_maps(x1, x2, attention_weights):
    x1 = np.ascontiguousarray(np.asarray(x1, dtype=np.float32))
    x2 = np.ascontiguousarray(np.asarray(x2, dtype=np.float32))
    w = np.asarray(attention_weights, dtype=np.float32)
    wb = np.ascontiguousarray(np.tile(w[None, :], (P, G)))
    x1s = np.split(x1, NCORES, axis=0)
    x2s = np.split(x2, NCORES, axis=0)
    return [
        {"x1": np.ascontiguousarray(x1s[c]), "x2": np.ascontiguousarray(x2s[c]), "wb": wb}
        for c in range(NCORES)
    ]


def run(x1, x2, attention_weights, trace=False, **trace_kwargs):
    """Run on 8 cores; returns ((fused, alpha), BassKernelResults)."""
    _ensure_concourse()
    from concourse.bass_utils import run_bass_kernel_spmd

    nc = _get_program()
    in_maps = make_in_maps(x1, x2, attention_weights)
    res = run_bass_kernel_spmd(
        nc, in_maps, core_ids=list(range(NCORES)), trace=trace, **trace_kwargs
    )
    fused = np.concatenate([res.results[c]["fused"] for c in range(NCORES)], axis=0)
    alpha = np.concatenate([res.results[c]["alpha"] for c in range(NCORES)], axis=0)
    return (fused, alpha), res


def kernel(x1, x2, attention_weights):
    (fused, alpha), _ = run(x1, x2, attention_weights, trace=False)
    return fused, alpha
